# revision 6
# baseline (speedup 1.0000x reference)
"""Trainium2 Bass kernel for DistanceClusterLoss.

loss = mean_i sum_{j != i} sign_ij * ||x_i - x_j||,  sign_ij = +1 if labels
match else -1, over x = preds.reshape(N, D), N=8192, D=200.

Cyclic half-band symmetry: every unordered pair {i, j} is computed exactly
once because each row i covers columns j in (i, i+4096] mod N (gap-4096
pairs are seen from both sides and get weight 1/2 each, applied as 0.5 in
the post-sqrt band-edge mask).

Sharding: 1024 rows per core, column-ROTATED operand copies per core so one
SPMD program serves all 8 cores.

Per [128, 512] tile:
  *  PE: ONE fp8e4 DoubleRow matmul (K=206 packed as [103, 2, .]) computes
     psum = d2/2 directly: rows 0..199 are x8_i * (-x8_j), rows 200..205
     carry s_i/2 and s_j/2 as 3-term fp8 residue splits (last term rounded
     UP so the diagonal lands in [0, 0.125] -- sqrt never sees a negative).
  *  ACT: dist = sqrt(2*psum), batched over up to 3 PSUM banks per
     instruction, bf16 out.
  *  band-edge tiles only: GPSIMD/DVE multiply dist by the post-sqrt mask
     (start: [j>i]; end: [gap<4096] + 0.5 at gap 4096).
  *  PE: mask matmul with bf16 [one-hot(labels) | ones] weights accumulates
     T[c, j] = sum_i [l_i = c] dist_ij and T[64, j] = sum_i dist_ij.
  *  DVE: multiply T by m2 (2*one-hot for c<64, -1 for the ones row) and
     reduce: yields sum_band (2*[l_i=l_j] - 1) * dist directly.

Host sums the per-core [65, 11] partials in float64: out = 2 * S / N.
"""

import sys

sys.path.insert(0, "/opt/trn_rl_repo")

import numpy as np
import ml_dtypes

N = 8192
D = 200
NCORES = 8
NB = N // NCORES        # 1024 rows per core
MT = NB // 128          # 8 row tiles per core
BAND = N // 2           # 4096
ROT = BAND + NB         # 5120 rotated columns used per core
RJT = ROT // 512        # 10 rotated column tiles
NCLS = 64
CW = NCLS + 1           # one-hot classes + all-ones row
KA = 206                # fp8 contraction rows: 200 dims + 3 row-aux + 3 col-aux
KP = KA // 2            # 103 partitions, DoubleRow packs 2 K-slices
CHUNK = 1024            # column split of the rotated operands (5 chunks)

_CACHE = {}
LAST_EXEC_NS = None


def _tile_kind(m, jt):
    """Return 'start' | 'mid' | 'end' | None for row-tile m at rotated
    column tile jt."""
    jt_s = m // 4
    k = jt - jt_s
    if k < 0 or k > 8:
        return None
    return "start" if k == 0 else ("end" if k == 8 else "mid")


def _build(reps=1, drop=frozenset()):
    drop = frozenset(drop)
    if ("nc", reps, drop) in _CACHE:
        return _CACHE[("nc", reps, drop)]

    import concourse.tile as tile
    from concourse import bacc, mybir

    f32 = mybir.dt.float32
    bf16 = mybir.dt.bfloat16
    fp8 = mybir.dt.float8e4

    nc = bacc.Bacc("TRN2", target_bir_lowering=False, debug=False)

    nch = ROT // CHUNK
    # rotated fp8 rhs, DoubleRow layout [KP, 2, CHUNK]
    d_xr = [
        nc.dram_tensor(f"xr{k}", [KP, 2 * CHUNK], fp8, kind="ExternalInput")
        for k in range(nch)
    ]
    # local fp8 lhsT, DoubleRow layout [KP, 2, NB]
    d_xl0 = nc.dram_tensor("xl0", [KP, 2 * 256], fp8, kind="ExternalInput")
    d_xl1 = nc.dram_tensor("xl1", [KP, 2 * (NB - 256)], fp8, kind="ExternalInput")
    d_m2 = [
        nc.dram_tensor(f"m2_{k}", [CW, CHUNK], bf16, kind="ExternalInput")
        for k in range(nch)
    ]
    d_co = nc.dram_tensor("co", [128, MT * CW], bf16, kind="ExternalInput")
    # post-sqrt band-edge masks, one [128, 512] block per (m % 4)
    d_ms = nc.dram_tensor("ms", [128, 4 * 512], bf16, kind="ExternalInput")
    d_me = nc.dram_tensor("me", [128, 4 * 512], bf16, kind="ExternalInput")
    d_accb = nc.dram_tensor("accb", [CW, RJT], f32, kind="ExternalOutput")
    if reps > 1:
        d_chkb = nc.dram_tensor("chkb", [CW, RJT], f32, kind="ExternalOutput")

    with tile.TileContext(nc) as tc:
        with (
            tc.tile_pool(name="singles", bufs=1) as singles,
            tc.tile_pool(name="pd", bufs=2, space="PSUM") as pd_pool,
            tc.tile_pool(name="pb", bufs=2, space="PSUM") as pb_pool,
            tc.tile_pool(name="dist", bufs=2) as distp,
            tc.tile_pool(name="work", bufs=4) as work,
            tc.tile_pool(name="prodp", bufs=2) as prodp,
        ):
            xl = singles.tile([KP, 2, NB], fp8, tag="xl")
            nc.gpsimd.dma_start(out=xl[:, :, 0:256], in_=d_xl0[:, :])
            xr = [None] * nch
            m2 = [None] * nch
            ms = me = co = None
            # chunk DMA order follows the jt processing order so compute
            # starts early and the rest streams behind it.
            for k in (1, 2, 0):
                xr[k] = singles.tile([KP, 2, CHUNK], fp8, tag=f"xr{k}", name=f"xr{k}")
                nc.gpsimd.dma_start(out=xr[k], in_=d_xr[k][:, :])
                if co is None:
                    nc.gpsimd.dma_start(out=xl[:, :, 256:NB], in_=d_xl1[:, :])
                    co = singles.tile([128, MT * CW], bf16, tag="co")
                    nc.gpsimd.dma_start(out=co, in_=d_co[:, :])
                    ms = singles.tile([128, 4 * 512], bf16, tag="ms")
                    nc.gpsimd.dma_start(out=ms, in_=d_ms[:, :])
                m2[k] = singles.tile([CW, CHUNK], bf16, tag=f"m2_{k}", name=f"m2s{k}")
                nc.gpsimd.dma_start(out=m2[k], in_=d_m2[k][:, :])
            me = singles.tile([128, 4 * 512], bf16, tag="me")
            nc.gpsimd.dma_start(out=me, in_=d_me[:, :])
            for k in (3, 4):
                xr[k] = singles.tile([KP, 2, CHUNK], fp8, tag=f"xr{k}", name=f"xr{k}")
                nc.gpsimd.dma_start(out=xr[k], in_=d_xr[k][:, :])
                m2[k] = singles.tile([CW, CHUNK], bf16, tag=f"m2_{k}", name=f"m2s{k}")
                nc.gpsimd.dma_start(out=m2[k], in_=d_m2[k][:, :])
            accb = singles.tile([CW, RJT], f32, tag="accb")
            if reps > 1:
                sumb = singles.tile([CW, RJT], f32, tag="sumb")
                nc.vector.memset(sumb, 0.0)

            JT_ORDER = (2, 3, 4, 1, 5, 0, 6, 8, 9, 7)
            for _rep in range(reps):
              for jt in JT_ORDER:
                if "body" in drop:
                    continue
                j0 = jt * 512
                jc, joff = j0 // CHUNK, j0 % CHUNK
                mlist = [m for m in range(MT) if _tile_kind(m, jt)]
                mlist.sort(key=lambda m: _tile_kind(m, jt) != "mid")
                pb = pb_pool.tile([CW, 512], f32)
                # groups of <=3 row-tiles share one multi-bank PSUM tile and
                # one batched ACT sqrt
                groups = [mlist[g : g + 3] for g in range(0, len(mlist), 3)]
                ci = 0
                for glist in groups:
                    gw = len(glist) * 512
                    pd = pd_pool.tile([128, gw], f32)
                    for gi, m in enumerate(glist):
                        i0 = m * 128
                        nc.tensor.matmul(
                            pd[:, gi * 512 : gi * 512 + 512],
                            lhsT=xl[:, :, i0 : i0 + 128],
                            rhs=xr[jc][:, :, joff : joff + 512],
                            start=True,
                            stop=True,
                            perf_mode=mybir.MatmulPerfMode.DoubleRow,
                        )
                    dist = distp.tile([128, gw], bf16, tag="dist")
                    nc.scalar.activation(
                        out=dist,
                        in_=pd,
                        func=mybir.ActivationFunctionType.Sqrt,
                        scale=2.0,
                    )
                    for gi, m in enumerate(glist):
                        kind = _tile_kind(m, jt)
                        dsl = dist[:, gi * 512 : gi * 512 + 512]
                        o4 = (m % 4) * 512
                        if kind == "start":
                            src = work.tile([128, 512], bf16, tag="mstart")
                            nc.vector.tensor_mul(src, dsl, ms[:, o4 : o4 + 512])
                        elif kind == "end":
                            src = work.tile([128, 512], bf16, tag="mend")
                            nc.vector.tensor_mul(src, dsl, me[:, o4 : o4 + 512])
                        else:
                            src = dsl
                        nc.tensor.matmul(
                            pb,
                            lhsT=co[:, m * CW : (m + 1) * CW],
                            rhs=src,
                            start=(ci == 0),
                            stop=(ci == len(mlist) - 1),
                        )
                        ci += 1
                prod = prodp.tile([CW, 512], f32, tag="prod")
                nc.vector.tensor_mul(prod, pb, m2[jc][:, joff : joff + 512])
                nc.vector.reduce_sum(
                    accb[:, jt : jt + 1], prod, axis=mybir.AxisListType.X
                )
              if "body" in drop:
                  nc.vector.memset(accb, 0.0)
              if reps > 1:
                  nc.vector.tensor_add(sumb, sumb, accb)
            nc.sync.dma_start(out=d_accb[:, :], in_=accb)
            if reps > 1:
                nc.sync.dma_start(out=d_chkb[:, :], in_=sumb)

    nc.compile()
    _CACHE[("nc", reps, drop)] = nc
    return nc


def _fp8(v):
    return np.asarray(v, np.float32).astype(ml_dtypes.float8_e4m3)


def _fp8_ceil(v):
    """fp8 round-up: nearest, then bump one ulp where the result fell short."""
    q = _fp8(v)
    qf = q.astype(np.float32)
    low = qf < v
    if low.any():
        # next representable value above qf: add half-step via int trick
        bumped = np.where(
            qf >= 0,
            (q.view(np.uint8) + 1).astype(np.uint8),
            (q.view(np.uint8) - 1).astype(np.uint8),
        ).view(ml_dtypes.float8_e4m3)
        q = np.where(low, bumped, q)
    return q.astype(ml_dtypes.float8_e4m3)


def _split3(v):
    """v ~ a1+a2+a3 in fp8 with a3 rounded UP so the sum is >= v."""
    a1 = _fp8(v)
    r1 = v - a1.astype(np.float32)
    a2 = _fp8(r1)
    r2 = r1 - a2.astype(np.float32)
    a3 = _fp8_ceil(r2 + 0.125)
    return a1, a2, a3


def _pack_dr(rows):
    """[KA, X] -> DoubleRow [KP, 2, X] (slice s holds K row s*KP + p)."""
    ka, x = rows.shape
    assert ka == KA
    return np.ascontiguousarray(
        rows.reshape(2, KP, x).transpose(1, 0, 2)
    )


def _prepare_inputs(preds, labels):
    x = np.ascontiguousarray(np.asarray(preds).reshape(N, D), dtype=np.float32)
    lab = np.asarray(labels).astype(np.int64)

    x8 = _fp8(x)                                   # [N, 200]
    x8f = x8.astype(np.float32)
    s = (x8f.astype(np.float64) ** 2).sum(-1).astype(np.float32)   # exact |x8|^2
    h = 0.5 * s
    a1, a2, a3 = _split3(h)                        # row-aux (lhsT side)
    c1, c2, c3 = _split3(h)                        # col-aux (rhs side)

    # lhsT rows [KA, N]: x8 dims, then row-aux, then ones to pick up col-aux
    lrows = np.zeros((KA, N), ml_dtypes.float8_e4m3)
    lrows[:D] = x8.T
    lrows[D] = a1
    lrows[D + 1] = a2
    lrows[D + 2] = a3
    lrows[D + 3 : D + 6] = _fp8(1.0)
    # rhs rows [KA, N]: -x8 dims, ones for row-aux, col-aux
    rrows = np.zeros((KA, N), ml_dtypes.float8_e4m3)
    rrows[:D] = _fp8(-x8f.T)
    rrows[D : D + 3] = _fp8(1.0)
    rrows[D + 3] = c1
    rrows[D + 4] = c2
    rrows[D + 5] = c3

    onehot = (lab[:, None] == np.arange(NCLS)[None, :]).astype(np.float32)
    m2p = np.empty((CW, N), np.float32)
    m2p[:NCLS] = 2.0 * onehot.T
    m2p[NCLS] = -1.0

    # post-sqrt band-edge masks: o = (m % 4) * 128
    p = np.arange(128)[:, None]
    f = np.arange(512)[None, :]
    ms = np.empty((128, 4 * 512), np.float32)
    me = np.empty((128, 4 * 512), np.float32)
    for q in range(4):
        o = q * 128
        ms[:, q * 512 : (q + 1) * 512] = (f > o + p).astype(np.float32)
        me[:, q * 512 : (q + 1) * 512] = np.where(
            f < o + p, 1.0, np.where(f == o + p, 0.5, 0.0)
        ).astype(np.float32)
    ms16 = ms.astype(ml_dtypes.bfloat16)
    me16 = me.astype(ml_dtypes.bfloat16)

    nch = ROT // CHUNK
    in_maps = []
    for c in range(NCORES):
        r0 = c * NB
        idx = (r0 + np.arange(ROT)) % N
        rr = _pack_dr(np.ascontiguousarray(rrows[:, idx]))   # [KP, 2, ROT]
        m2_rot = m2p[:, idx]
        im = {"ms": ms16, "me": me16}
        for k in range(nch):
            sl = slice(k * CHUNK, (k + 1) * CHUNK)
            im[f"xr{k}"] = np.ascontiguousarray(rr[:, :, sl]).reshape(KP, 2 * CHUNK)
            im[f"m2_{k}"] = np.ascontiguousarray(m2_rot[:, sl]).astype(
                ml_dtypes.bfloat16
            )
        ll = _pack_dr(np.ascontiguousarray(lrows[:, r0 : r0 + NB]))
        im["xl0"] = np.ascontiguousarray(ll[:, :, 0:256]).reshape(KP, 2 * 256)
        im["xl1"] = np.ascontiguousarray(ll[:, :, 256:NB]).reshape(
            KP, 2 * (NB - 256)
        )
        coh = np.empty((128, MT * CW), np.float32)
        for m in range(MT):
            rsl = slice(r0 + m * 128, r0 + (m + 1) * 128)
            coh[:, m * CW : m * CW + NCLS] = onehot[rsl]
            coh[:, m * CW + NCLS] = 1.0
        im["co"] = coh.astype(ml_dtypes.bfloat16)
        in_maps.append(im)
    return in_maps


def kernel(preds, labels):
    global LAST_EXEC_NS
    import os

    from concourse.bass_utils import run_bass_kernel_spmd

    nc = _build()
    in_maps = _prepare_inputs(preds, labels)
    trace = os.environ.get("BASSK_TRACE") == "1"
    res = run_bass_kernel_spmd(
        nc, in_maps, core_ids=list(range(NCORES)), trace=trace
    )
    if trace:
        LAST_EXEC_NS = res.exec_time_ns

    S = 0.0
    for c in range(NCORES):
        S += float(res.results[c]["accb"].sum(dtype=np.float64))
    out = 2.0 * S / N
    return np.asarray(out, dtype=np.float32)


# revision 8
# speedup vs baseline: 1.0235x; 1.0235x over previous
"""Trainium2 Bass kernel for DistanceClusterLoss.

loss = mean_i sum_{j != i} sign_ij * ||x_i - x_j||,  sign_ij = +1 if labels
match else -1, over x = preds.reshape(N, D), N=8192, D=200.

Cyclic half-band symmetry: every unordered pair {i, j} is computed exactly
once because each row i covers columns j in (i, i+4096] mod N (gap-4096
pairs are seen from both sides and get weight 1/2 each, applied as 0.5 in
the post-sqrt band-edge mask).

Sharding: 1024 rows per core, column-ROTATED operand copies per core so one
SPMD program serves all 8 cores.

Per [128, 512] tile:
  *  PE: ONE fp8e4 DoubleRow matmul (K=206 packed as [103, 2, .]) computes
     psum = d2/2 directly: rows 0..199 are x8_i * (-x8_j), rows 200..205
     carry s_i/2 and s_j/2 as 3-term fp8 residue splits (last term rounded
     UP so the diagonal lands in [0, 0.125] -- sqrt never sees a negative).
  *  ACT: dist = sqrt(2*psum), batched over up to 3 PSUM banks per
     instruction, bf16 out.
  *  band-edge tiles only: GPSIMD/DVE multiply dist by the post-sqrt mask
     (start: [j>i]; end: [gap<4096] + 0.5 at gap 4096).
  *  PE: mask matmul with bf16 [one-hot(labels) | ones] weights accumulates
     T[c, j] = sum_i [l_i = c] dist_ij and T[64, j] = sum_i dist_ij.
  *  DVE: multiply T by m2 (2*one-hot for c<64, -1 for the ones row) and
     reduce: yields sum_band (2*[l_i=l_j] - 1) * dist directly.

Host sums the per-core [65, 11] partials in float64: out = 2 * S / N.
"""

import sys

sys.path.insert(0, "/opt/trn_rl_repo")

import numpy as np
import ml_dtypes

N = 8192
D = 200
NCORES = 8
NB = N // NCORES        # 1024 rows per core
MT = NB // 128          # 8 row tiles per core
BAND = N // 2           # 4096
ROT = BAND + NB         # 5120 rotated columns used per core
RJT = ROT // 512        # 10 rotated column tiles
NCLS = 64
CW = NCLS + 1           # one-hot classes + all-ones row
KA = 206                # fp8 contraction rows: 200 dims + 3 row-aux + 3 col-aux
KP = KA // 2            # 103 partitions, DoubleRow packs 2 K-slices
CHUNK = 1024            # column split of the rotated operands (5 chunks)

_CACHE = {}
LAST_EXEC_NS = None


def _tile_kind(m, jt):
    """Return 'start' | 'mid' | 'end' | None for row-tile m at rotated
    column tile jt."""
    jt_s = m // 4
    k = jt - jt_s
    if k < 0 or k > 8:
        return None
    return "start" if k == 0 else ("end" if k == 8 else "mid")


def _build(reps=1, drop=frozenset()):
    drop = frozenset(drop)
    if ("nc", reps, drop) in _CACHE:
        return _CACHE[("nc", reps, drop)]

    import concourse.tile as tile
    from concourse import bacc, mybir

    f32 = mybir.dt.float32
    bf16 = mybir.dt.bfloat16
    fp8 = mybir.dt.float8e4

    nc = bacc.Bacc("TRN2", target_bir_lowering=False, debug=False)

    nch = ROT // CHUNK
    # rotated fp8 rhs, DoubleRow layout [KP, 2, CHUNK]
    d_xr = [
        nc.dram_tensor(f"xr{k}", [KP, 2 * CHUNK], fp8, kind="ExternalInput")
        for k in range(nch)
    ]
    # local fp8 lhsT, DoubleRow layout [KP, 2, NB]
    d_xl0 = nc.dram_tensor("xl0", [KP, 2 * 256], fp8, kind="ExternalInput")
    d_xl1 = nc.dram_tensor("xl1", [KP, 2 * (NB - 256)], fp8, kind="ExternalInput")
    d_m2 = nc.dram_tensor("m2", [CW, ROT], bf16, kind="ExternalInput")
    d_co = nc.dram_tensor("co", [128, MT * CW], bf16, kind="ExternalInput")
    # post-sqrt band-edge masks ms|me, one [128, 512] block per (m % 4) each
    d_msme = nc.dram_tensor("msme", [128, 8 * 512], bf16, kind="ExternalInput")
    d_accb = nc.dram_tensor("accb", [CW, RJT], f32, kind="ExternalOutput")
    if reps > 1:
        d_chkb = nc.dram_tensor("chkb", [CW, RJT], f32, kind="ExternalOutput")

    with tile.TileContext(nc) as tc:
        with (
            tc.tile_pool(name="singles", bufs=1) as singles,
            tc.tile_pool(name="pd", bufs=2, space="PSUM") as pd_pool,
            tc.tile_pool(name="pb", bufs=2, space="PSUM") as pb_pool,
            tc.tile_pool(name="dist", bufs=2) as distp,
            tc.tile_pool(name="work", bufs=4) as work,
            tc.tile_pool(name="prodp", bufs=2) as prodp,
        ):
            xl = singles.tile([KP, 2, NB], fp8, tag="xl")
            xr = [None] * nch
            for k in range(nch):
                xr[k] = singles.tile([KP, 2, CHUNK], fp8, tag=f"xr{k}", name=f"xr{k}")
            co = singles.tile([128, MT * CW], bf16, tag="co")
            msme = singles.tile([128, 8 * 512], bf16, tag="msme")
            m2t = singles.tile([CW, ROT], bf16, tag="m2")
            ms = msme[:, 0 : 4 * 512]
            me = msme[:, 4 * 512 : 8 * 512]
            m2 = [m2t[:, k * CHUNK : (k + 1) * CHUNK] for k in range(nch)]
            # kickoffs split across the SP and DVE sequencers; order follows
            # the jt processing order so compute starts early.
            nc.sync.dma_start(out=xl[:, :, 0:256], in_=d_xl0[:, :])
            nc.sync.dma_start(out=xr[1], in_=d_xr[1][:, :])
            nc.sync.dma_start(out=xl[:, :, 256:NB], in_=d_xl1[:, :])
            nc.sync.dma_start(out=co, in_=d_co[:, :])
            nc.gpsimd.dma_start(out=xr[2], in_=d_xr[2][:, :])
            nc.gpsimd.dma_start(out=m2t, in_=d_m2[:, :])
            nc.gpsimd.dma_start(out=xr[0], in_=d_xr[0][:, :])
            nc.gpsimd.dma_start(out=msme, in_=d_msme[:, :])
            nc.gpsimd.dma_start(out=xr[3], in_=d_xr[3][:, :])
            nc.gpsimd.dma_start(out=xr[4], in_=d_xr[4][:, :])
            accb = singles.tile([CW, RJT], f32, tag="accb")
            if reps > 1:
                sumb = singles.tile([CW, RJT], f32, tag="sumb")
                nc.vector.memset(sumb, 0.0)

            JT_ORDER = (2, 3, 4, 1, 5, 0, 6, 8, 9, 7)
            for _rep in range(reps):
              for jt in JT_ORDER:
                if "body" in drop:
                    continue
                j0 = jt * 512
                jc, joff = j0 // CHUNK, j0 % CHUNK
                mlist = [m for m in range(MT) if _tile_kind(m, jt)]
                mlist.sort(key=lambda m: _tile_kind(m, jt) != "mid")
                pb = pb_pool.tile([CW, 512], f32)
                # groups of <=3 row-tiles share one multi-bank PSUM tile and
                # one batched ACT sqrt
                groups = [mlist[g : g + 3] for g in range(0, len(mlist), 3)]
                ci = 0
                for glist in groups:
                    gw = len(glist) * 512
                    pd = pd_pool.tile([128, gw], f32)
                    for gi, m in enumerate(glist):
                        i0 = m * 128
                        nc.tensor.matmul(
                            pd[:, gi * 512 : gi * 512 + 512],
                            lhsT=xl[:, :, i0 : i0 + 128],
                            rhs=xr[jc][:, :, joff : joff + 512],
                            start=True,
                            stop=True,
                            perf_mode=mybir.MatmulPerfMode.DoubleRow,
                        )
                    dist = distp.tile([128, gw], bf16, tag="dist")
                    nc.scalar.activation(
                        out=dist,
                        in_=pd,
                        func=mybir.ActivationFunctionType.Sqrt,
                        scale=2.0,
                    )
                    for gi, m in enumerate(glist):
                        kind = _tile_kind(m, jt)
                        dsl = dist[:, gi * 512 : gi * 512 + 512]
                        o4 = (m % 4) * 512
                        if kind == "start":
                            src = work.tile([128, 512], bf16, tag="mstart")
                            nc.vector.tensor_mul(src, dsl, ms[:, o4 : o4 + 512])
                        elif kind == "end":
                            src = work.tile([128, 512], bf16, tag="mend")
                            nc.vector.tensor_mul(src, dsl, me[:, o4 : o4 + 512])
                        else:
                            src = dsl
                        nc.tensor.matmul(
                            pb,
                            lhsT=co[:, m * CW : (m + 1) * CW],
                            rhs=src,
                            start=(ci == 0),
                            stop=(ci == len(mlist) - 1),
                        )
                        ci += 1
                prod = prodp.tile([CW, 512], f32, tag="prod")
                nc.vector.tensor_mul(prod, pb, m2[jc][:, joff : joff + 512])
                nc.vector.reduce_sum(
                    accb[:, jt : jt + 1], prod, axis=mybir.AxisListType.X
                )
              if "body" in drop:
                  nc.vector.memset(accb, 0.0)
              if reps > 1:
                  nc.vector.tensor_add(sumb, sumb, accb)
            nc.sync.dma_start(out=d_accb[:, :], in_=accb)
            if reps > 1:
                nc.sync.dma_start(out=d_chkb[:, :], in_=sumb)

    nc.compile()
    _CACHE[("nc", reps, drop)] = nc
    return nc


def _fp8(v):
    return np.asarray(v, np.float32).astype(ml_dtypes.float8_e4m3)


def _fp8_ceil(v):
    """fp8 round-up: nearest, then bump one ulp where the result fell short."""
    q = _fp8(v)
    qf = q.astype(np.float32)
    low = qf < v
    if low.any():
        # next representable value above qf: add half-step via int trick
        bumped = np.where(
            qf >= 0,
            (q.view(np.uint8) + 1).astype(np.uint8),
            (q.view(np.uint8) - 1).astype(np.uint8),
        ).view(ml_dtypes.float8_e4m3)
        q = np.where(low, bumped, q)
    return q.astype(ml_dtypes.float8_e4m3)


def _split3(v):
    """v ~ a1+a2+a3 in fp8 with a3 rounded UP so the sum is >= v."""
    a1 = _fp8(v)
    r1 = v - a1.astype(np.float32)
    a2 = _fp8(r1)
    r2 = r1 - a2.astype(np.float32)
    a3 = _fp8_ceil(r2 + 0.125)
    return a1, a2, a3


def _pack_dr(rows):
    """[KA, X] -> DoubleRow [KP, 2, X] (slice s holds K row s*KP + p)."""
    ka, x = rows.shape
    assert ka == KA
    return np.ascontiguousarray(
        rows.reshape(2, KP, x).transpose(1, 0, 2)
    )


def _prepare_inputs(preds, labels):
    x = np.ascontiguousarray(np.asarray(preds).reshape(N, D), dtype=np.float32)
    lab = np.asarray(labels).astype(np.int64)

    x8 = _fp8(x)                                   # [N, 200]
    x8f = x8.astype(np.float32)
    s = (x8f.astype(np.float64) ** 2).sum(-1).astype(np.float32)   # exact |x8|^2
    h = 0.5 * s
    a1, a2, a3 = _split3(h)                        # row-aux (lhsT side)
    c1, c2, c3 = _split3(h)                        # col-aux (rhs side)

    # lhsT rows [KA, N]: x8 dims, then row-aux, then ones to pick up col-aux
    lrows = np.zeros((KA, N), ml_dtypes.float8_e4m3)
    lrows[:D] = x8.T
    lrows[D] = a1
    lrows[D + 1] = a2
    lrows[D + 2] = a3
    lrows[D + 3 : D + 6] = _fp8(1.0)
    # rhs rows [KA, N]: -x8 dims, ones for row-aux, col-aux
    rrows = np.zeros((KA, N), ml_dtypes.float8_e4m3)
    rrows[:D] = _fp8(-x8f.T)
    rrows[D : D + 3] = _fp8(1.0)
    rrows[D + 3] = c1
    rrows[D + 4] = c2
    rrows[D + 5] = c3

    onehot = (lab[:, None] == np.arange(NCLS)[None, :]).astype(np.float32)
    m2p = np.empty((CW, N), np.float32)
    m2p[:NCLS] = 2.0 * onehot.T
    m2p[NCLS] = -1.0

    # post-sqrt band-edge masks: o = (m % 4) * 128
    p = np.arange(128)[:, None]
    f = np.arange(512)[None, :]
    ms = np.empty((128, 4 * 512), np.float32)
    me = np.empty((128, 4 * 512), np.float32)
    for q in range(4):
        o = q * 128
        ms[:, q * 512 : (q + 1) * 512] = (f > o + p).astype(np.float32)
        me[:, q * 512 : (q + 1) * 512] = np.where(
            f < o + p, 1.0, np.where(f == o + p, 0.5, 0.0)
        ).astype(np.float32)
    ms16 = ms.astype(ml_dtypes.bfloat16)
    me16 = me.astype(ml_dtypes.bfloat16)

    nch = ROT // CHUNK
    in_maps = []
    for c in range(NCORES):
        r0 = c * NB
        idx = (r0 + np.arange(ROT)) % N
        rr = _pack_dr(np.ascontiguousarray(rrows[:, idx]))   # [KP, 2, ROT]
        m2_rot = m2p[:, idx]
        im = {"msme": np.ascontiguousarray(np.concatenate([ms16, me16], axis=1))}
        im["m2"] = np.ascontiguousarray(m2_rot).astype(ml_dtypes.bfloat16)
        for k in range(nch):
            sl = slice(k * CHUNK, (k + 1) * CHUNK)
            im[f"xr{k}"] = np.ascontiguousarray(rr[:, :, sl]).reshape(KP, 2 * CHUNK)
        ll = _pack_dr(np.ascontiguousarray(lrows[:, r0 : r0 + NB]))
        im["xl0"] = np.ascontiguousarray(ll[:, :, 0:256]).reshape(KP, 2 * 256)
        im["xl1"] = np.ascontiguousarray(ll[:, :, 256:NB]).reshape(
            KP, 2 * (NB - 256)
        )
        coh = np.empty((128, MT * CW), np.float32)
        for m in range(MT):
            rsl = slice(r0 + m * 128, r0 + (m + 1) * 128)
            coh[:, m * CW : m * CW + NCLS] = onehot[rsl]
            coh[:, m * CW + NCLS] = 1.0
        im["co"] = coh.astype(ml_dtypes.bfloat16)
        in_maps.append(im)
    return in_maps


def kernel(preds, labels):
    global LAST_EXEC_NS
    import os

    from concourse.bass_utils import run_bass_kernel_spmd

    nc = _build()
    in_maps = _prepare_inputs(preds, labels)
    trace = os.environ.get("BASSK_TRACE") == "1"
    res = run_bass_kernel_spmd(
        nc, in_maps, core_ids=list(range(NCORES)), trace=trace
    )
    if trace:
        LAST_EXEC_NS = res.exec_time_ns

    S = 0.0
    for c in range(NCORES):
        S += float(res.results[c]["accb"].sum(dtype=np.float64))
    out = 2.0 * S / N
    return np.asarray(out, dtype=np.float32)
